# revision 5
# baseline (speedup 1.0000x reference)
"""Trainium2 Bass kernel for ConceptNBM N-ary (gnn_message_passing).

Strategy:
  - Fold BatchNorm (eval) into each Linear -> 4x fused Linear+ReLU per order.
  - Fold conv_w (grouped conv k=1) and cls_w into per-tuple rank-1 matrices
    M2[t] = conv_w[t] (x) cls_w[:,t]  [100 x 10]; then
    out[c,b] = sum_t M2[t].T @ h3_t[:, b] accumulated in PSUM (col-packed
    2 tuples -> partition strips 0-9 / 32-41; host sums the strips).
  - Work unit = a PAIR of tuples x 512 batch (psum tiles [*, 1024]).
    Layer-0 is a K=3 matmul over rows [x_i; x_j; 1] with weights
    [W0'[:,0]; W0'[:,1]; b0'] (bias folded via the ones row).
  - fp16 operands everywhere on the PE (216 ns / 512-col matmul, warm),
    fp32 PSUM accumulation; ~1e-3 total relative error.
  - Duplicate tuples (possible with randint-generated idx2) are deduped
    host-side by summing their M2 matrices.
  - Core 0 handles order-1 tuples as its first tiles with weight set "A";
    all other cores get A == B == order-2 params, so one compiled program
    serves all 8 cores (SPMD, data-only variation).
"""

import numpy as np

import concourse.bass as bass
import concourse.mybir as mybir
import concourse.tile as tile
from concourse import bacc
from concourse.bass_utils import run_bass_kernel_spmd

N_CORES = 8
B = 512
NB = 100             # NUM_BASES
NCLS = 10
BN_EPS = 1e-5

F32 = mybir.dt.float32
F16 = mybir.dt.float16

TRACE = False        # set by test harness to capture an NTFF profile
LAST_RESULT = None   # BassKernelResults of the most recent run

_CACHE = {}          # (NT, A_COUNT) -> compiled Bacc


def _fold_bn(p, order):
    """Fold BN(eval) into Linear per layer. Returns dict of device blobs."""
    out = {}
    Ws, bs = [], []
    for i in range(4):
        W = np.asarray(p[f"w{i}"], dtype=np.float64)
        b = np.asarray(p[f"b{i}"], dtype=np.float64)
        g = np.asarray(p[f"g{i}"], dtype=np.float64)
        beta = np.asarray(p[f"beta{i}"], dtype=np.float64)
        m = np.asarray(p[f"m{i}"], dtype=np.float64)
        v = np.asarray(p[f"v{i}"], dtype=np.float64)
        s = g / np.sqrt(v + BN_EPS)
        Ws.append(W * s[:, None])
        bs.append(s * (b - m) + beta)

    # L0: lhsT [3, 256] rows = [W0'[:,0], W0'[:,1] (or 0), b0']
    l0 = np.zeros((3, 256), dtype=np.float64)
    l0[0] = Ws[0][:, 0]
    if order == 2:
        l0[1] = Ws[0][:, 1]
    l0[2] = bs[0]
    out["l0"] = l0.astype(np.float16)

    # L1: lhsT chunks [128, 2, 128] flattened to [128, 256]
    w1t = Ws[1].T.astype(np.float16)                      # [256, 128]
    out["w1"] = np.ascontiguousarray(
        w1t.reshape(2, 128, 128).transpose(1, 0, 2)
    ).reshape(128, 256)
    out["w2"] = np.ascontiguousarray(Ws[2].T.astype(np.float16))  # [128,128]
    out["w3"] = np.ascontiguousarray(Ws[3].T.astype(np.float16))  # [128,100]
    out["b1"] = bs[1].astype(np.float32).reshape(128, 1)
    out["b2"] = bs[2].astype(np.float32).reshape(128, 1)
    out["b3"] = bs[3].astype(np.float32).reshape(NB, 1)
    return out


def _build_program(NT, A_COUNT):
    """NT = tiles (tuples incl padding) per core, even. First A_COUNT tiles
    use weight set A (order-1 on core 0), rest use set B. A_COUNT even."""
    NP = NT // 2
    nc = bacc.Bacc("TRN2", target_bir_lowering=False, debug=False, num_devices=N_CORES)

    xg_d = nc.dram_tensor("xg", [NP, 3, 2 * B], F16, kind="ExternalInput").ap()
    m2_d = nc.dram_tensor("m2", [NB, NT * NCLS], F16, kind="ExternalInput").ap()
    wsets = {}
    for tag in ("A", "B"):
        wsets[tag] = {
            "l0": nc.dram_tensor(f"w{tag}_l0", [3, 256], F16, kind="ExternalInput").ap(),
            "w1": nc.dram_tensor(f"w{tag}_w1", [128, 256], F16, kind="ExternalInput").ap(),
            "w2": nc.dram_tensor(f"w{tag}_w2", [128, 128], F16, kind="ExternalInput").ap(),
            "w3": nc.dram_tensor(f"w{tag}_w3", [128, NB], F16, kind="ExternalInput").ap(),
            "b1": nc.dram_tensor(f"w{tag}_b1", [128, 1], F32, kind="ExternalInput").ap(),
            "b2": nc.dram_tensor(f"w{tag}_b2", [128, 1], F32, kind="ExternalInput").ap(),
            "b3": nc.dram_tensor(f"w{tag}_b3", [NB, 1], F32, kind="ExternalInput").ap(),
        }
    out_d = nc.dram_tensor("out", [2, NCLS, B], F32, kind="ExternalOutput").ap()

    Relu = mybir.ActivationFunctionType.Relu
    add_op = mybir.AluOpType.add
    max_op = mybir.AluOpType.max

    def evict(engine, dst, src, bias):
        """relu(src + bias) -> dst on the given engine ('act' or 'dve')."""
        if engine == "act":
            nc.scalar.activation(dst, src, Relu, bias=(0.0 if bias is None else bias))
        else:
            if bias is None:
                nc.vector.tensor_scalar_max(dst, src, 0.0)
            else:
                nc.vector.tensor_scalar(dst, src, bias, 0.0, add_op, max_op)

    with tile.TileContext(nc) as tc:
        with tc.tile_pool(name="const", bufs=1) as const, \
             tc.tile_pool(name="xgp", bufs=3) as xgp, \
             tc.tile_pool(name="hp", bufs=2) as hp, \
             tc.tile_pool(name="ps", bufs=1, space="PSUM") as ps, \
             tc.tile_pool(name="pc", bufs=1, space="PSUM") as pc:

            # --- resident constants ---
            m2_sb = const.tile([NB, NT * NCLS], F16)
            nc.sync.dma_start(m2_sb[:], m2_d)
            wsb = {}
            for tag in ("A", "B"):
                w = wsets[tag]
                wsb[tag] = {
                    "l0": const.tile([3, 256], F16, name=f"l0{tag}"),
                    "w1": const.tile([128, 256], F16, name=f"w1{tag}"),
                    "w2": const.tile([128, 128], F16, name=f"w2{tag}"),
                    "w3": const.tile([128, NB], F16, name=f"w3{tag}"),
                    "b1": const.tile([128, 1], F32, name=f"b1{tag}"),
                    "b2": const.tile([128, 1], F32, name=f"b2{tag}"),
                    "b3": const.tile([NB, 1], F32, name=f"b3{tag}"),
                }
                for k in wsb[tag]:
                    nc.sync.dma_start(wsb[tag][k][:], w[k])

            cls_ps = pc.tile([42, B], F32)   # strips: rows 0-9 (t0), 32-41 (t1)

            # Software-pipelined emission. Per-pair stages:
            #   A(k): xg dma + L0 (4 MM) + E0a/E0b + L1 (4 MM) + E1
            #   Bst(k): L2 (2 MM) + E2          [l2 shares l1's PSUM slot]
            #   C(k): l3 half0 MM + E3a
            #   D(k): cls0 MM; l3 half1 MM + E3b
            #   E(k): cls1 MM
            # Emitted per iteration: Bst(k-1), A(k), C(k-2), D(k-3), E(k-4)
            # so every wait references work emitted >= ~1 iteration earlier.
            PREF = 4
            xg_tiles = {}
            h0_t = {}
            h1_t = {}
            h2_t = {}
            h3_t = {}

            def dma_xg(k):
                if k >= NP:
                    return
                xg_sb = xgp.tile([3, 2 * B], F16, tag="xg", name="xg")
                nc.sync.dma_start(xg_sb[:], xg_d[k])
                xg_tiles[k] = xg_sb

            def stage_A(k):
                W = wsb["A"] if k < A_COUNT // 2 else wsb["B"]
                dma_xg(k + PREF)
                xg_sb = xg_tiles.pop(k)
                l0a = ps.tile([128, 2 * B], F32, tag="l0a", name="l0a")
                l0b = ps.tile([128, 2 * B], F32, tag="l0b", name="l0b")
                for h in range(2):
                    nc.tensor.matmul(l0a[:, h * B:(h + 1) * B], W["l0"][:, 0:128],
                                     xg_sb[:, h * B:(h + 1) * B], start=True, stop=True)
                for h in range(2):
                    nc.tensor.matmul(l0b[:, h * B:(h + 1) * B], W["l0"][:, 128:256],
                                     xg_sb[:, h * B:(h + 1) * B], start=True, stop=True)
                h0 = hp.tile([128, 2, 2 * B], F16, tag="h0", name="h0", bufs=3)
                evict("dve", h0[:, 0, :], l0a[:], None)     # E0a
                evict("act", h0[:, 1, :], l0b[:], None)     # E0b
                l1 = ps.tile([128, 2 * B], F32, tag="l12", name="l1")
                for h in range(2):
                    sl = slice(h * B, (h + 1) * B)
                    nc.tensor.matmul(l1[:, sl], W["w1"][:, 0:128], h0[:, 0, sl],
                                     start=True, stop=False)
                    nc.tensor.matmul(l1[:, sl], W["w1"][:, 128:256], h0[:, 1, sl],
                                     start=False, stop=True)
                h1 = hp.tile([128, 2 * B], F16, tag="h1", name="h1", bufs=3)
                evict("dve", h1[:], l1[:], W["b1"][:])      # E1
                h0_t[k] = h0
                h1_t[k] = h1

            def stage_B(k):
                W = wsb["A"] if k < A_COUNT // 2 else wsb["B"]
                h1 = h1_t.pop(k)
                l2 = ps.tile([128, 2 * B], F32, tag="l12", name="l2")
                for h in range(2):
                    sl = slice(h * B, (h + 1) * B)
                    nc.tensor.matmul(l2[:, sl], W["w2"][:], h1[:, sl], start=True, stop=True)
                h2 = hp.tile([128, 2 * B], F16, tag="h2", name="h2", bufs=4)
                evict("act", h2[:], l2[:], W["b2"][:])      # E2
                h2_t[k] = h2
                h0_t.pop(k, None)

            def stage_C(k):
                W = wsb["A"] if k < A_COUNT // 2 else wsb["B"]
                h2 = h2_t[k]
                l3 = ps.tile([NB, B], F32, tag="l3", name="l3")
                nc.tensor.matmul(l3[:], W["w3"][:], h2[:, 0:B], start=True, stop=True)
                h3 = hp.tile([NB, B], F16, tag="h3_0", name="h3", bufs=4)
                evict("dve", h3[:], l3[:], W["b3"][:])      # E3a
                h3_t[(k, 0)] = h3

            def stage_D(k):
                W = wsb["A"] if k < A_COUNT // 2 else wsb["B"]
                # cls for half 0 (waits E3a(k), emitted last iteration)
                t = 2 * k
                nc.tensor.matmul(
                    cls_ps[0:NCLS, :],
                    m2_sb[:, t * NCLS:(t + 1) * NCLS],
                    h3_t.pop((k, 0))[:],
                    start=(k == 0), stop=(k == NP - 1),
                    skip_group_check=True,
                )
                h2 = h2_t.pop(k)
                l3 = ps.tile([NB, B], F32, tag="l3", name="l3")
                nc.tensor.matmul(l3[:], W["w3"][:], h2[:, B:2 * B], start=True, stop=True)
                h3 = hp.tile([NB, B], F16, tag="h3_1", name="h3", bufs=4)
                evict("act", h3[:], l3[:], W["b3"][:])      # E3b
                h3_t[(k, 1)] = h3

            def stage_E(k):
                t = 2 * k + 1
                nc.tensor.matmul(
                    cls_ps[32:32 + NCLS, :],
                    m2_sb[:, t * NCLS:(t + 1) * NCLS],
                    h3_t.pop((k, 1))[:],
                    start=(k == 0), stop=(k == NP - 1),
                    skip_group_check=True,
                )

            for k in range(PREF):
                dma_xg(k)
            for k in range(NP + 4):
                if 1 <= k <= NP:
                    stage_B(k - 1)
                if k < NP:
                    stage_A(k)
                if 2 <= k <= NP + 1:
                    stage_C(k - 2)
                if 3 <= k <= NP + 2:
                    stage_D(k - 3)
                if 4 <= k <= NP + 3:
                    stage_E(k - 4)

            out_sb = const.tile([42, B], F32)
            nc.scalar.activation(out_sb[:], cls_ps[:],
                                 mybir.ActivationFunctionType.Copy)
            nc.sync.dma_start(out_d[0], out_sb[0:NCLS, :])
            nc.sync.dma_start(out_d[1], out_sb[32:32 + NCLS, :])

    nc.compile()
    return nc


def _get_program(NT, A_COUNT):
    key = (NT, A_COUNT)
    if key not in _CACHE:
        _CACHE[key] = _build_program(NT, A_COUNT)
    return _CACHE[key]


def kernel(x, idx1, idx2, params_o1, params_o2, conv_w, conv_b, cls_w, cls_b):
    global LAST_RESULT
    x = np.asarray(x, dtype=np.float32)
    idx1 = np.asarray(idx1, dtype=np.int32)
    idx2 = np.asarray(idx2, dtype=np.int32)
    conv_w = np.asarray(conv_w, dtype=np.float32)
    conv_b = np.asarray(conv_b, dtype=np.float32)
    cls_w = np.asarray(cls_w, dtype=np.float32)
    cls_b = np.asarray(cls_b, dtype=np.float32)
    T1 = idx1.shape[0]
    T2 = idx2.shape[0]

    pA = _fold_bn(params_o1, order=1)
    pB = _fold_bn(params_o2, order=2)

    # M2[t] = conv_w[t] (x) cls_w[:, t] -> [T, NB, NCLS]
    m2_all = (conv_w[:, :, None] * cls_w.T[:, None, :]).astype(np.float32)

    # ---- dedupe tuples (order-1 by i; order-2 by (i,j)), summing M2 ----
    # unique units: list of (i, j, is_o2); m2 accumulated per unit
    units = {}
    order_keys = []
    for t in range(T1):
        k = ("o1", int(idx1[t, 0]))
        if k not in units:
            units[k] = [int(idx1[t, 0]), 0, np.zeros((NB, NCLS), np.float32)]
            order_keys.append(k)
        units[k][2] += m2_all[t]
    n_o1 = len(order_keys)
    for t in range(T2):
        k = ("o2", int(idx2[t, 0]), int(idx2[t, 1]))
        if k not in units:
            units[k] = [int(idx2[t, 0]), int(idx2[t, 1]), np.zeros((NB, NCLS), np.float32)]
            order_keys.append(k)
        units[k][2] += m2_all[T1 + t]

    # pad order-1 units to even count (weight-set switch is at pair granularity)
    o1_units = order_keys[:n_o1]
    o2_units = order_keys[n_o1:]
    U1 = len(o1_units)
    if U1 % 2 == 1:
        U1 += 1
        o1_units = o1_units + [None]   # dummy o1 unit (m2 = 0)
    U = U1 + len(o2_units)
    # NT tiles per core, even, NT >= U1 (all o1 units must fit on core 0)
    NT = -(-U // N_CORES)
    NT = max(NT + (NT % 2), U1 + (U1 % 2), 2)
    worklist = o1_units + o2_units + [None] * (N_CORES * NT - U)
    A_COUNT = U1

    in_maps = []
    for g in range(N_CORES):
        chunk = worklist[g * NT:(g + 1) * NT]
        xg = np.zeros((NT, 3, B), dtype=np.float16)
        m2 = np.zeros((NT, NB, NCLS), dtype=np.float32)
        for n, k in enumerate(chunk):
            if k is None:
                xg[n, 2, :] = 1.0
                continue
            i, j, m2u = units[k]
            xg[n, 0, :] = x[:, i].astype(np.float16)
            if k[0] == "o2":
                xg[n, 1, :] = x[:, j].astype(np.float16)
            xg[n, 2, :] = 1.0
            m2[n] = m2u
        # pair layout [NP, 3, 2B]
        xgp = np.ascontiguousarray(
            xg.reshape(NT // 2, 2, 3, B).transpose(0, 2, 1, 3)
        ).reshape(NT // 2, 3, 2 * B)
        m2f = np.ascontiguousarray(
            m2.transpose(1, 0, 2)
        ).reshape(NB, NT * NCLS).astype(np.float16)

        pa = pA if g == 0 else pB
        im = {"xg": xgp, "m2": m2f}
        for kk in ("l0", "w1", "w2", "w3", "b1", "b2", "b3"):
            im[f"wA_{kk}"] = pa[kk]
            im[f"wB_{kk}"] = pB[kk]
        in_maps.append(im)

    nc = _get_program(NT, A_COUNT)
    res = run_bass_kernel_spmd(nc, in_maps, list(range(N_CORES)), trace=bool(TRACE))
    LAST_RESULT = res

    partial = np.zeros((NCLS, B), dtype=np.float64)
    for g in range(N_CORES):
        partial += res.results[g]["out"].astype(np.float64).sum(axis=0)

    bias = cls_w.astype(np.float64) @ conv_b.astype(np.float64) + cls_b.astype(np.float64)
    out = partial.T + bias[None, :]
    return out.astype(np.float32)


# revision 6
# speedup vs baseline: 1.4559x; 1.4559x over previous
"""Trainium2 Bass kernel for ConceptNBM N-ary (gnn_message_passing).

Strategy:
  - Fold BatchNorm (eval) into each Linear -> 4x fused Linear+ReLU per order.
  - Fold conv_w (grouped conv k=1) and cls_w into per-tuple rank-1 matrices
    M2[t] = conv_w[t] (x) cls_w[:,t]  [100 x 10]; then
    out[c,b] = sum_t M2[t].T @ h3_t[:, b] accumulated in PSUM (col-packed
    2 tuples -> partition strips 0-9 / 32-41; host sums the strips).
  - Work unit = a PAIR of tuples x 512 batch (psum tiles [*, 1024]).
    Layer-0 is a K=3 matmul over rows [x_i; x_j; 1] with weights
    [W0'[:,0]; W0'[:,1]; b0'] (bias folded via the ones row).
  - fp16 operands everywhere on the PE (216 ns / 512-col matmul, warm),
    fp32 PSUM accumulation; ~1e-3 total relative error.
  - Duplicate tuples (possible with randint-generated idx2) are deduped
    host-side by summing their M2 matrices.
  - Core 0 handles order-1 tuples as its first tiles with weight set "A";
    all other cores get A == B == order-2 params, so one compiled program
    serves all 8 cores (SPMD, data-only variation).
"""

import numpy as np

import concourse.bass as bass
import concourse.mybir as mybir
import concourse.tile as tile
from concourse import bacc
from concourse.bass_utils import run_bass_kernel_spmd

N_CORES = 8
B = 512
NB = 100             # NUM_BASES
NCLS = 10
BN_EPS = 1e-5

F32 = mybir.dt.float32
F16 = mybir.dt.float16

TRACE = False        # set by test harness to capture an NTFF profile
LAST_RESULT = None   # BassKernelResults of the most recent run

_CACHE = {}          # (NT, A_COUNT) -> compiled Bacc


def _fold_bn(p, order):
    """Fold BN(eval) into Linear per layer. Returns dict of device blobs."""
    out = {}
    Ws, bs = [], []
    for i in range(4):
        W = np.asarray(p[f"w{i}"], dtype=np.float64)
        b = np.asarray(p[f"b{i}"], dtype=np.float64)
        g = np.asarray(p[f"g{i}"], dtype=np.float64)
        beta = np.asarray(p[f"beta{i}"], dtype=np.float64)
        m = np.asarray(p[f"m{i}"], dtype=np.float64)
        v = np.asarray(p[f"v{i}"], dtype=np.float64)
        s = g / np.sqrt(v + BN_EPS)
        Ws.append(W * s[:, None])
        bs.append(s * (b - m) + beta)

    # L0: lhsT [3, 256] rows = [W0'[:,0], W0'[:,1] (or 0), b0']
    l0 = np.zeros((3, 256), dtype=np.float64)
    l0[0] = Ws[0][:, 0]
    if order == 2:
        l0[1] = Ws[0][:, 1]
    l0[2] = bs[0]
    out["l0"] = l0.astype(np.float16)

    # L1: lhsT chunks [128, 2, 128] flattened to [128, 256]
    w1t = Ws[1].T.astype(np.float16)                      # [256, 128]
    out["w1"] = np.ascontiguousarray(
        w1t.reshape(2, 128, 128).transpose(1, 0, 2)
    ).reshape(128, 256)
    out["w2"] = np.ascontiguousarray(Ws[2].T.astype(np.float16))  # [128,128]
    out["w3"] = np.ascontiguousarray(Ws[3].T.astype(np.float16))  # [128,100]
    out["b1"] = bs[1].astype(np.float32).reshape(128, 1)
    out["b2"] = bs[2].astype(np.float32).reshape(128, 1)
    out["b3"] = bs[3].astype(np.float32).reshape(NB, 1)
    return out


def _build_program(NT, A_COUNT):
    """NT = tiles (tuples incl padding) per core, even. First A_COUNT tiles
    use weight set A (order-1 on core 0), rest use set B. A_COUNT even."""
    NP = NT // 2
    nc = bacc.Bacc("TRN2", target_bir_lowering=False, debug=False, num_devices=N_CORES)

    xg_d = nc.dram_tensor("xg", [NP, 3, 2 * B], F16, kind="ExternalInput").ap()
    m2_d = nc.dram_tensor("m2", [NB, NT * NCLS], F16, kind="ExternalInput").ap()
    wsets = {}
    for tag in ("A", "B"):
        wsets[tag] = {
            "l0": nc.dram_tensor(f"w{tag}_l0", [3, 256], F16, kind="ExternalInput").ap(),
            "w1": nc.dram_tensor(f"w{tag}_w1", [128, 256], F16, kind="ExternalInput").ap(),
            "w2": nc.dram_tensor(f"w{tag}_w2", [128, 128], F16, kind="ExternalInput").ap(),
            "w3": nc.dram_tensor(f"w{tag}_w3", [128, NB], F16, kind="ExternalInput").ap(),
            "b1": nc.dram_tensor(f"w{tag}_b1", [128, 1], F32, kind="ExternalInput").ap(),
            "b2": nc.dram_tensor(f"w{tag}_b2", [128, 1], F32, kind="ExternalInput").ap(),
            "b3": nc.dram_tensor(f"w{tag}_b3", [NB, 1], F32, kind="ExternalInput").ap(),
        }
    out_d = nc.dram_tensor("out", [2, NCLS, B], F32, kind="ExternalOutput").ap()

    Relu = mybir.ActivationFunctionType.Relu
    add_op = mybir.AluOpType.add
    max_op = mybir.AluOpType.max

    def evict(engine, dst, src, bias):
        """relu(src + bias) -> dst on the given engine ('act' or 'dve')."""
        if engine == "act":
            nc.scalar.activation(dst, src, Relu, bias=(0.0 if bias is None else bias))
        else:
            if bias is None:
                nc.vector.tensor_scalar_max(dst, src, 0.0)
            else:
                nc.vector.tensor_scalar(dst, src, bias, 0.0, add_op, max_op)

    with tile.TileContext(nc) as tc:
        with tc.tile_pool(name="const", bufs=1) as const, \
             tc.tile_pool(name="xgp", bufs=3) as xgp, \
             tc.tile_pool(name="hp", bufs=2) as hp, \
             tc.tile_pool(name="ps", bufs=1, space="PSUM") as ps, \
             tc.tile_pool(name="pc", bufs=1, space="PSUM") as pc:

            # --- resident constants ---
            m2_sb = const.tile([NB, NT * NCLS], F16)
            nc.sync.dma_start(m2_sb[:], m2_d)
            wsb = {}
            for tag in ("A", "B"):
                w = wsets[tag]
                wsb[tag] = {
                    "l0": const.tile([3, 256], F16, name=f"l0{tag}"),
                    "w1": const.tile([128, 256], F16, name=f"w1{tag}"),
                    "w2": const.tile([128, 128], F16, name=f"w2{tag}"),
                    "w3": const.tile([128, NB], F16, name=f"w3{tag}"),
                    "b1": const.tile([128, 1], F32, name=f"b1{tag}"),
                    "b2": const.tile([128, 1], F32, name=f"b2{tag}"),
                    "b3": const.tile([NB, 1], F32, name=f"b3{tag}"),
                }
                for k in wsb[tag]:
                    nc.sync.dma_start(wsb[tag][k][:], w[k])

            cls_ps = pc.tile([42, B], F32)   # strips: rows 0-9 (t0), 32-41 (t1)

            # PE warm-up burst: ~20 dense matmuls unthrottle the HAM clock
            # gate (K=4/8 -> 8/8) before the pipeline starts.
            wps = ps.tile([128, B], F32, tag="l3", name="wps")
            for _ in range(20):
                nc.tensor.matmul(wps[:], m2_sb[0:100, 0:128], m2_sb[0:100, 0:B],
                                 start=True, stop=True, skip_group_check=True)

            # Software-pipelined emission. Stages per pair k (all PSUM tiles
            # one bank except l1/l2):
            #   A1(k): L0 half0 (2 MM into l0a/l0b banks) + E0a0/E0b0
            #   A2(k): L0 half1 (2 MM, same banks after eviction) + E0a1/E0b1
            #   B(k):  L1 (4 MM) + E1
            #   C(k):  L2 (2 MM) + E2
            #   D(k):  l3 half0 (1 MM) + E3a
            #   E(k):  l3 half1 (1 MM) + E3b
            #   F(k):  cls0+cls1 (2 MM, col strips 0/32 run concurrently)
            # Emission per iteration: A1(k), B(k-1), A2(k), C(k-2), D(k-3),
            # E(k-4), F(k-5) -- every wait references work >=~1 iteration old.
            PREF = 4
            xg_tiles = {}
            h0_t = {}
            h1_t = {}
            h2_t = {}
            h3_t = {}

            def eng(k, i):
                """Alternate eviction engines per pair parity for balance."""
                return ("dve", "act")[(k + i) % 2]

            def dma_xg(k):
                if k >= NP:
                    return
                xg_sb = xgp.tile([3, 2 * B], F16, tag="xg", name="xg", bufs=PREF + 2)
                nc.sync.dma_start(xg_sb[:], xg_d[k])
                xg_tiles[k] = xg_sb

            def wset(k):
                return wsb["A"] if k < A_COUNT // 2 else wsb["B"]

            def stage_A(k, h):
                W = wset(k)
                if h == 0:
                    dma_xg(k + PREF)
                    h0_t[k] = hp.tile([128, 2, 2 * B], F16, tag="h0", name="h0", bufs=3)
                xg_sb = xg_tiles[k]
                h0 = h0_t[k]
                sl = slice(h * B, (h + 1) * B)
                l0a = ps.tile([128, B], F32, tag="l0a", name="l0a")
                l0b = ps.tile([128, B], F32, tag="l0b", name="l0b")
                nc.tensor.matmul(l0a[:], W["l0"][:, 0:128], xg_sb[:, sl],
                                 start=True, stop=True)
                nc.tensor.matmul(l0b[:], W["l0"][:, 128:256], xg_sb[:, sl],
                                 start=True, stop=True)
                evict(eng(k, h), h0[:, 0, sl], l0a[:], None)        # E0a_h
                evict(eng(k, h + 1), h0[:, 1, sl], l0b[:], None)    # E0b_h
                if h == 1:
                    xg_tiles.pop(k)

            def stage_B(k):
                W = wset(k)
                h0 = h0_t.pop(k)
                l1 = ps.tile([128, 2 * B], F32, tag="l1", name="l1")
                for h in range(2):
                    sl = slice(h * B, (h + 1) * B)
                    nc.tensor.matmul(l1[:, sl], W["w1"][:, 0:128], h0[:, 0, sl],
                                     start=True, stop=False)
                    nc.tensor.matmul(l1[:, sl], W["w1"][:, 128:256], h0[:, 1, sl],
                                     start=False, stop=True)
                h1 = hp.tile([128, 2 * B], F16, tag="h1", name="h1", bufs=3)
                evict(eng(k, 0), h1[:], l1[:], W["b1"][:])          # E1
                h1_t[k] = h1

            def stage_C(k):
                W = wset(k)
                h1 = h1_t.pop(k)
                l2 = ps.tile([128, 2 * B], F32, tag="l2", name="l2")
                for h in range(2):
                    sl = slice(h * B, (h + 1) * B)
                    nc.tensor.matmul(l2[:, sl], W["w2"][:], h1[:, sl], start=True, stop=True)
                h2 = hp.tile([128, 2 * B], F16, tag="h2", name="h2", bufs=4)
                evict(eng(k, 1), h2[:], l2[:], W["b2"][:])          # E2
                h2_t[k] = h2

            def stage_DE(k, h):
                W = wset(k)
                h2 = h2_t[k] if h == 0 else h2_t.pop(k)
                l3 = ps.tile([NB, B], F32, tag="l3", name="l3")
                nc.tensor.matmul(l3[:], W["w3"][:], h2[:, h * B:(h + 1) * B],
                                 start=True, stop=True)
                h3 = hp.tile([NB, B], F16, tag=f"h3_{h}", name="h3", bufs=4)
                evict(eng(k, h), h3[:], l3[:], W["b3"][:])          # E3a/E3b
                h3_t[(k, h)] = h3

            def stage_F(k):
                for h in range(2):
                    t = 2 * k + h
                    nc.tensor.matmul(
                        cls_ps[32 * h:32 * h + NCLS, :],
                        m2_sb[:, t * NCLS:(t + 1) * NCLS],
                        h3_t.pop((k, h))[:],
                        start=(k == 0), stop=(k == NP - 1),
                        skip_group_check=True,
                    )

            for k in range(PREF):
                dma_xg(k)
            for k in range(NP + 6):
                if k < NP:
                    stage_A(k, 0)
                if 1 <= k <= NP:
                    stage_B(k - 1)
                if k < NP:
                    stage_A(k, 1)
                if 2 <= k <= NP + 1:
                    stage_C(k - 2)
                if 3 <= k <= NP + 2:
                    stage_DE(k - 3, 0)
                if 4 <= k <= NP + 3:
                    stage_DE(k - 4, 1)
                if 5 <= k <= NP + 4:
                    stage_F(k - 5)

            out_sb = const.tile([42, B], F32)
            nc.scalar.activation(out_sb[:], cls_ps[:],
                                 mybir.ActivationFunctionType.Copy)
            nc.sync.dma_start(out_d[0], out_sb[0:NCLS, :])
            nc.sync.dma_start(out_d[1], out_sb[32:32 + NCLS, :])

    nc.compile()
    return nc


def _get_program(NT, A_COUNT):
    key = (NT, A_COUNT)
    if key not in _CACHE:
        _CACHE[key] = _build_program(NT, A_COUNT)
    return _CACHE[key]


def kernel(x, idx1, idx2, params_o1, params_o2, conv_w, conv_b, cls_w, cls_b):
    global LAST_RESULT
    x = np.asarray(x, dtype=np.float32)
    idx1 = np.asarray(idx1, dtype=np.int32)
    idx2 = np.asarray(idx2, dtype=np.int32)
    conv_w = np.asarray(conv_w, dtype=np.float32)
    conv_b = np.asarray(conv_b, dtype=np.float32)
    cls_w = np.asarray(cls_w, dtype=np.float32)
    cls_b = np.asarray(cls_b, dtype=np.float32)
    T1 = idx1.shape[0]
    T2 = idx2.shape[0]

    pA = _fold_bn(params_o1, order=1)
    pB = _fold_bn(params_o2, order=2)

    # M2[t] = conv_w[t] (x) cls_w[:, t] -> [T, NB, NCLS]
    m2_all = (conv_w[:, :, None] * cls_w.T[:, None, :]).astype(np.float32)

    # ---- dedupe tuples (order-1 by i; order-2 by (i,j)), summing M2 ----
    # unique units: list of (i, j, is_o2); m2 accumulated per unit
    units = {}
    order_keys = []
    for t in range(T1):
        k = ("o1", int(idx1[t, 0]))
        if k not in units:
            units[k] = [int(idx1[t, 0]), 0, np.zeros((NB, NCLS), np.float32)]
            order_keys.append(k)
        units[k][2] += m2_all[t]
    n_o1 = len(order_keys)
    for t in range(T2):
        k = ("o2", int(idx2[t, 0]), int(idx2[t, 1]))
        if k not in units:
            units[k] = [int(idx2[t, 0]), int(idx2[t, 1]), np.zeros((NB, NCLS), np.float32)]
            order_keys.append(k)
        units[k][2] += m2_all[T1 + t]

    # pad order-1 units to even count (weight-set switch is at pair granularity)
    o1_units = order_keys[:n_o1]
    o2_units = order_keys[n_o1:]
    U1 = len(o1_units)
    if U1 % 2 == 1:
        U1 += 1
        o1_units = o1_units + [None]   # dummy o1 unit (m2 = 0)
    U = U1 + len(o2_units)
    # NT tiles per core, even, NT >= U1 (all o1 units must fit on core 0)
    NT = -(-U // N_CORES)
    NT = max(NT + (NT % 2), U1 + (U1 % 2), 2)
    worklist = o1_units + o2_units + [None] * (N_CORES * NT - U)
    A_COUNT = U1

    in_maps = []
    for g in range(N_CORES):
        chunk = worklist[g * NT:(g + 1) * NT]
        xg = np.zeros((NT, 3, B), dtype=np.float16)
        m2 = np.zeros((NT, NB, NCLS), dtype=np.float32)
        for n, k in enumerate(chunk):
            if k is None:
                xg[n, 2, :] = 1.0
                continue
            i, j, m2u = units[k]
            xg[n, 0, :] = x[:, i].astype(np.float16)
            if k[0] == "o2":
                xg[n, 1, :] = x[:, j].astype(np.float16)
            xg[n, 2, :] = 1.0
            m2[n] = m2u
        # pair layout [NP, 3, 2B]
        xgp = np.ascontiguousarray(
            xg.reshape(NT // 2, 2, 3, B).transpose(0, 2, 1, 3)
        ).reshape(NT // 2, 3, 2 * B)
        m2f = np.ascontiguousarray(
            m2.transpose(1, 0, 2)
        ).reshape(NB, NT * NCLS).astype(np.float16)

        pa = pA if g == 0 else pB
        im = {"xg": xgp, "m2": m2f}
        for kk in ("l0", "w1", "w2", "w3", "b1", "b2", "b3"):
            im[f"wA_{kk}"] = pa[kk]
            im[f"wB_{kk}"] = pB[kk]
        in_maps.append(im)

    nc = _get_program(NT, A_COUNT)
    res = run_bass_kernel_spmd(nc, in_maps, list(range(N_CORES)), trace=bool(TRACE))
    LAST_RESULT = res

    partial = np.zeros((NCLS, B), dtype=np.float64)
    for g in range(N_CORES):
        partial += res.results[g]["out"].astype(np.float64).sum(axis=0)

    bias = cls_w.astype(np.float64) @ conv_b.astype(np.float64) + cls_b.astype(np.float64)
    out = partial.T + bias[None, :]
    return out.astype(np.float32)
